# revision 56
# baseline (speedup 1.0000x reference)
"""GCN encoder kernel for 8 Trainium2 NeuronCores (Bass/Tile, SPMD).

Strategy (dst-sharded graph parallel, per sharding hint):
  - Nodes are degree-sorted and padded to NPAD = 392 tiles of 128; tiles go
    round-robin to the 8 cores so every core sees the same per-position
    chunk-count profile (SPMD: one program, 8 in_maps).
  - Aggregation is linear, so each GCN layer is computed as
    (aggregate) @ W.T; layers 1 and 2 share ONE aggregation of h.
  - agg0 (over x): the host expands x*dinv[src] into a padded per-(node,slot)
    edge-feature stream (pure data movement / sharding prep); the device
    reduces it with PE matmuls against a constant identity (PSUM scatter-add).
  - agg1 (over h): the device gathers h rows with dma_gather (4 SWDGE
    queues round-robin, buffers deep enough for 2 tiles in flight), builds
    exact 0/1 one-hot S matrices ON-CHIP via DVE is_equal against a resident
    iota tile (dst indices stream in as a tiny resident f16 array), and
    scatter-adds with PE matmuls: psum[f,d] += G.T @ S.
  - h is exchanged in TWO AllGather collectives (first 25 tiles, last 24),
    each issued as soon as its half of h is ready -> mostly hidden under
    phase A.  Side effect: each gathered tensor has <32768 rows, so int16
    gather indices need no lo/hi range splitting.
  - All gather indices + dst streams are SBUF-resident (loaded once).
  - Symmetric normalization (dinv = 1/sqrt(deg+1)) is folded into host-side
    scale arrays and a per-partition output scale; self-loops are ordinary
    edges.  Outputs are written f16 and upcast on the host.
"""
import os
import sys

sys.path.insert(0, "/opt/trn_rl_repo")

import numpy as np

N, E, DIN, DH = 50000, 1600000, 128, 128
NCORES = 8
NPAD = ((N + 1023) // 1024) * 1024   # 50176 = 392 tiles of 128
TILES = NPAD // 128
TPC = TILES // NCORES                # positions (tiles) per core
NPC = TPC * 128                      # node rows per core
KA = 18                              # tiles in first AllGather half
KB = TPC - KA                        # tiles in second half
NRA = NCORES * KA * 128              # 18432 rows in h_fullA (< 32768)
NRB = NCORES * KB * 128              # 31744 rows in h_fullB (< 32768)


def _wrap_idx16(a):
    """dma_gather index layout: idx i -> [i%16, i//16], replicated 8x."""
    n = len(a)
    w = np.zeros((16, n // 16), np.int16)
    w[np.arange(n) % 16, np.arange(n) // 16] = a
    return np.tile(w, (8, 1))


def _build_kernel(CCA, CCB, CK, CA, has_b0, has_b12):
    """Build the SPMD Tile program. CCA/CCB/CK/CA are per-position chunk
    counts (compile-time constants, shared by all cores)."""
    import concourse.bass as bass  # noqa: F401
    import concourse.tile as tile
    from concourse import bacc, mybir

    f32, f16, i16 = mybir.dt.float32, mybir.dt.float16, mybir.dt.int16
    f8 = mybir.dt.float8e4
    SCA, SCK = sum(CA), sum(CK)
    SA, SB = sum(CCA), sum(CCB)
    CAmax, CKmax = max(CA), max(CK)

    nc = bacc.Bacc(None, target_bir_lowering=False, debug=False,
                   num_swdge_queues=4)

    xe_d = nc.dram_tensor("xe", [128, SCA * 128], f8, kind="ExternalInput")
    m1_d = nc.dram_tensor("m1", [TPC, 128, 128], f16, kind="ExternalInput")
    ia_d = nc.dram_tensor("ia", [128, SA * 8], i16, kind="ExternalInput")
    ib_d = nc.dram_tensor("ib", [128, SB * 8], i16, kind="ExternalInput")
    dk_d = nc.dram_tensor("dk", [128, SCK], f16, kind="ExternalInput")
    dinv_d = nc.dram_tensor("dinvp", [128, TPC], f32, kind="ExternalInput")
    ident_d = nc.dram_tensor("ident", [128, 128], f16, kind="ExternalInput")
    ident8_d = nc.dram_tensor("ident82", [128, 256], f8, kind="ExternalInput")
    iota_d = nc.dram_tensor("iota", [128, 128], f16, kind="ExternalInput")
    w0_d = nc.dram_tensor("w0t", [128, 128], f16, kind="ExternalInput")
    w1_d = nc.dram_tensor("w1t", [128, 128], f32, kind="ExternalInput")
    w2_d = nc.dram_tensor("w2t", [128, 128], f32, kind="ExternalInput")
    if has_b0:
        m2_d = nc.dram_tensor("m2", [TPC, 128, 128], f16, kind="ExternalInput")
    if has_b12:
        b1_d = nc.dram_tensor("b1b", [128, 128], f32, kind="ExternalInput")
        b2_d = nc.dram_tensor("b2b", [128, 128], f32, kind="ExternalInput")
    o1_d = nc.dram_tensor("o1", [TPC, 128, 128], f16, kind="ExternalOutput")
    o2_d = nc.dram_tensor("o2", [TPC, 128, 128], f16, kind="ExternalOutput")

    with tile.TileContext(nc) as tc:
        with (
            tc.tile_pool(name="const", bufs=1) as cpool,
            tc.tile_pool(name="acc", bufs=1) as apool,
            tc.tile_pool(name="dram", bufs=1, space="DRAM") as dpool,
        ):
            ident_sb = cpool.tile([128, 128], f16)
            nc.scalar.dma_start(ident_sb[:], ident_d[:])
            ident8_sb = cpool.tile([128, 256], f8)
            nc.scalar.dma_start(ident8_sb[:], ident8_d[:])
            iota_sb = cpool.tile([128, 128], f16)
            nc.scalar.dma_start(iota_sb[:], iota_d[:])
            w0_sb = cpool.tile([128, 128], f16)
            nc.scalar.dma_start(w0_sb[:], w0_d[:])
            w1_sb = cpool.tile([128, 128], f32)
            nc.scalar.dma_start(w1_sb[:], w1_d[:])
            w2_sb = cpool.tile([128, 128], f32)
            nc.scalar.dma_start(w2_sb[:], w2_d[:])
            dinv_sb = cpool.tile([128, TPC], f32)
            nc.scalar.dma_start(dinv_sb[:], dinv_d[:])
            ia_sb = cpool.tile([128, SA * 8], i16)
            ib_sb = cpool.tile([128, SB * 8], i16)
            dk_sb = cpool.tile([128, SCK], f16)
            if has_b12:
                b1_sb = cpool.tile([128, 128], f32)
                nc.scalar.dma_start(b1_sb[:], b1_d[:])
                b2_sb = cpool.tile([128, 128], f32)
                nc.scalar.dma_start(b2_sb[:], b2_d[:])

            h_sbA = apool.tile([128, KA, 128], f16)
            h_sbB = apool.tile([128, KB, 128], f16)
            m1r_sb = apool.tile([128, TPC, 128], f16)
            nc.scalar.dma_start(
                m1r_sb[:], m1_d[:].rearrange("t p f -> p t f"))

            h_locA = dpool.tile([KA, 128, 128], f16)
            h_locB = dpool.tile([KB, 128, 128], f16)
            h_fullA = dpool.tile([NRA, 128], f16, addr_space="Shared")
            h_fullB = dpool.tile([NRB, 128], f16, addr_space="Shared")

            # ---------------- Phase A: agg0 + h ----------------
            with (
                tc.tile_pool(name="xe", bufs=3) as xpool,
                tc.tile_pool(name="meta", bufs=3) as mpool,
                tc.tile_pool(name="y", bufs=3) as ypool,
                tc.tile_pool(name="ps", bufs=2, space="PSUM") as ppool,
                tc.tile_pool(name="ps2", bufs=2, space="PSUM") as ppool2,
            ):
                # the W0 matmul of tile k-1 is emitted AFTER tile k's psum
                # accumulation: the PE then never waits on the scalar copy
                # and the instruction stream stays dense (phase A is
                # PE-issue-bound, not FLOP-bound).
                def w0_tail(k, y_sb):
                    ps_h = ppool2.tile([128, 128], f32, tag="ph")
                    nc.tensor.matmul(ps_h[:], y_sb[:], w0_sb[:],
                                     start=True, stop=True)  # [n, f2]
                    tmp = ypool.tile([128, 128], f32, tag="tmp")
                    nc.vector.tensor_tensor(tmp[:], ps_h[:], m1r_sb[:, k, :],
                                            mybir.AluOpType.mult)
                    if has_b0:
                        m2_sb = mpool.tile([128, 128], f16, tag="m2")
                        nc.sync.dma_start(m2_sb[:], m2_d[k])
                        nc.vector.tensor_tensor(tmp[:], tmp[:], m2_sb[:],
                                                mybir.AluOpType.add)
                    if k < KA:
                        nc.scalar.activation(h_sbA[:, k, :], tmp[:],
                                             mybir.ActivationFunctionType.Relu)
                    else:
                        nc.scalar.activation(h_sbB[:, k - KA, :], tmp[:],
                                             mybir.ActivationFunctionType.Relu)

                acol = 0
                pend = None              # (k, y_sb) awaiting its W0 tail
                for k in range(TPC):
                    ca = CA[k]           # even by construction
                    ca2 = ca // 2
                    gt = xpool.tile([128, CAmax // 2, 256], f8, tag="gt")
                    nc.sync.dma_start(
                        gt[:, 0:ca2, :].rearrange("p c f -> p (c f)"),
                        xe_d[:, acol * 128:(acol + ca) * 128])
                    acol += ca

                    # fp8e4 DoubleRow: each matmul contracts TWO slots
                    # (lhsT = [slot2j | slot2j+1], rhs = [I | I]), halving
                    # the PE instruction count.
                    ps_a = ppool.tile([128, 128], f32, tag="pa")
                    id2 = ident8_sb[:].rearrange("p (two f) -> p two f", two=2)
                    for j in range(ca2):
                        nc.tensor.matmul(ps_a[:],
                                         gt[:, j, :].rearrange(
                                             "p (two f) -> p two f", two=2),
                                         id2,
                                         start=(j == 0), stop=(j == ca2 - 1),
                                         perf_mode=mybir.MatmulPerfMode.DoubleRow)
                    y_sb = ypool.tile([128, 128], f16, tag="y0")
                    nc.scalar.copy(y_sb[:], ps_a[:])        # [f, d] raw sums
                    if pend is not None:
                        w0_tail(*pend)
                    pend = (k, y_sb)

                    if k == 1:
                        nc.scalar.dma_start(ia_sb[:], ia_d[:])
                        nc.scalar.dma_start(ib_sb[:], ib_d[:])
                        nc.scalar.dma_start(dk_sb[:], dk_d[:])
                    if k == KA:
                        # tile KA-1's tail ran above, so h_sbA is complete
                        nc.sync.dma_start(
                            h_locA[:].rearrange("t p f -> p t f"), h_sbA[:])
                        nc.gpsimd.collective_compute(
                            "AllGather", mybir.AluOpType.bypass,
                            replica_groups=[list(range(NCORES))],
                            ins=[h_locA[:]], outs=[h_fullA[:]],
                        )
                w0_tail(*pend)
                nc.sync.dma_start(
                    h_locB[:].rearrange("t p f -> p t f"), h_sbB[:])

            # ---------------- Phase B: agg1 + outputs ----------------
            # Separate buffer rings for the A-half and B-half gathers, with
            # gather emission lagged behind buffer release so the gpsimd
            # queue never head-of-line blocks: each dma_gather's buffer was
            # freed two tile-periods earlier.  Queues rotate per call.
            RA, RB = 10, 6
            PREA, PREB = 10, 4
            qctr = [0]
            offA = np.cumsum([0] + CCA).tolist()
            offB = np.cumsum([0] + CCB).tolist()
            CCAmax, CCBmax = max(CCA), max(CCB)
            GAt = [None] * TPC
            GBt = [None] * TPC
            with (
                tc.tile_pool(name="ga", bufs=RA) as gapool,
                tc.tile_pool(name="gb", bufs=RB) as gbpool,
                tc.tile_pool(name="s", bufs=2) as spool,
                tc.tile_pool(name="y2", bufs=2) as ypool2,
                tc.tile_pool(name="o", bufs=2) as opool,
                tc.tile_pool(name="psb", bufs=2, space="PSUM") as ppoolb,
                tc.tile_pool(name="psb2", bufs=2, space="PSUM") as ppoolb2,
            ):
                def a_gather(k):
                    cca = CCA[k]
                    GA = gapool.tile([128, CCAmax, 128], f16, tag="GA")
                    GAt[k] = GA
                    nc.gpsimd.dma_gather(GA[:, 0:cca, :], h_fullA[:],
                                         ia_sb[:, offA[k] * 8:(offA[k] + cca) * 8],
                                         cca * 128, cca * 128, 128,
                                         elem_step=128, single_packet=False,
                                         queue_num=qctr[0] % 4)
                    qctr[0] += 1

                def b_gather(k):
                    ccb = CCB[k]
                    GB = gbpool.tile([128, CCBmax, 128], f16, tag="GB")
                    GBt[k] = GB
                    nc.gpsimd.dma_gather(GB[:, 0:ccb, :], h_fullB[:],
                                         ib_sb[:, offB[k] * 8:(offB[k] + ccb) * 8],
                                         ccb * 128, ccb * 128, 128,
                                         elem_step=128, single_packet=False,
                                         queue_num=qctr[0] % 4)
                    qctr[0] += 1

                for k in range(min(PREA, TPC)):
                    a_gather(k)
                nc.gpsimd.collective_compute(
                    "AllGather", mybir.AluOpType.bypass,
                    replica_groups=[list(range(NCORES))],
                    ins=[h_locB[:]], outs=[h_fullB[:]],
                )
                for k in range(min(PREB, TPC)):
                    b_gather(k)

                kcol = 0
                for k in range(TPC):
                    cca, ccb, ck = CCA[k], CCB[k], CK[k]
                    if k % 2 == 0:
                        for kk in (k + PREA, k + PREA + 1):
                            if kk < TPC and kk >= PREA:
                                a_gather(kk)
                        for kk in (k + PREB, k + PREB + 1):
                            if kk < TPC and kk >= PREB:
                                b_gather(kk)

                    S = spool.tile([128, CKmax, 128], f16, tag="S")
                    iota_bc = iota_sb[:].rearrange(
                        "p (o f) -> p o f", o=1).broadcast_to([128, ck, 128])
                    dk_bc = dk_sb[:, kcol:kcol + ck].rearrange(
                        "p (c o) -> p c o", o=1).broadcast_to([128, ck, 128])
                    nc.vector.tensor_tensor(
                        S[:, 0:ck, :], iota_bc, dk_bc,
                        mybir.AluOpType.is_equal)
                    kcol += ck

                    GA, GB = GAt[k], GBt[k]
                    ps_b = ppoolb.tile([128, 128], f32, tag="pb")
                    for c in range(cca):
                        nc.tensor.matmul(ps_b[:], GA[:, c, :], S[:, c, :],
                                         start=(c == 0), stop=False)
                    for c in range(ccb):
                        nc.tensor.matmul(ps_b[:], GB[:, c, :],
                                         S[:, cca + c, :],
                                         start=False, stop=(c == ccb - 1))
                    y2 = ypool2.tile([128, 128], f32, tag="y2")
                    nc.scalar.copy(y2[:], ps_b[:])          # [f, d] raw sums

                    ps_o1 = ppoolb2.tile([128, 128], f32, tag="po")
                    nc.tensor.matmul(ps_o1[:], y2[:], w1_sb[:],
                                     start=True, stop=True)
                    ps_o2 = ppoolb2.tile([128, 128], f32, tag="po")
                    nc.tensor.matmul(ps_o2[:], y2[:], w2_sb[:],
                                     start=True, stop=True)

                    o1t = opool.tile([128, 128], f16, tag="o1")
                    nc.scalar.activation(o1t[:], ps_o1[:],
                                         mybir.ActivationFunctionType.Copy,
                                         scale=dinv_sb[:, k:k + 1])
                    o2t = opool.tile([128, 128], f16, tag="o2")
                    nc.scalar.activation(o2t[:], ps_o2[:],
                                         mybir.ActivationFunctionType.Copy,
                                         scale=dinv_sb[:, k:k + 1])
                    if has_b12:
                        nc.vector.tensor_tensor(o1t[:], o1t[:], b1_sb[:],
                                                mybir.AluOpType.add)
                        nc.vector.tensor_tensor(o2t[:], o2t[:], b2_sb[:],
                                                mybir.AluOpType.add)
                    nc.sync.dma_start(o1_d[k], o1t[:])
                    nc.sync.dma_start(o2_d[k], o2t[:])

    nc.compile()
    return nc


def kernel(x, edge_index, drop_mask, W0, b0, W1, b1, W2, b2, **_):
    import ml_dtypes
    from concourse.bass_utils import run_bass_kernel_spmd

    x = np.asarray(x, np.float32)
    edge_index = np.asarray(edge_index)
    drop_mask = np.asarray(drop_mask, np.float32)
    W0, W1, W2 = (np.asarray(w, np.float32) for w in (W0, W1, W2))
    b0, b1, b2 = (np.asarray(b, np.float32) for b in (b0, b1, b2))
    src0, dst0 = edge_index[0].astype(np.int64), edge_index[1].astype(np.int64)

    # ---- normalization / permutation (host: index-side preprocessing) ----
    deg = np.bincount(dst0, minlength=N).astype(np.float32) + 1.0
    dinv = 1.0 / np.sqrt(deg)

    perm = np.argsort(-deg, kind="stable")           # position -> node id
    pos = np.empty(N, np.int64)                      # node id -> position
    pos[perm] = np.arange(N)

    # self loops as ordinary edges
    src_a = np.concatenate([src0, np.arange(N)])
    dst_a = np.concatenate([dst0, np.arange(N)])
    sp = pos[src_a]
    dp = pos[dst_a]

    # h storage row of a source position, split across the two AllGathers:
    #   tile t = p//128 -> core t%8, per-core tile index t//8
    #   first KA per-core tiles -> h_fullA, rest -> h_fullB
    st = sp // 128
    sk = st // NCORES
    in_a = sk < KA
    hrow = np.where(
        in_a,
        (st % NCORES) * (KA * 128) + sk * 128 + (sp % 128),
        (st % NCORES) * (KB * 128) + (sk - KA) * 128 + (sp % 128))

    tile_of = dp // 128
    core_of = tile_of % NCORES
    kpos_of = tile_of // NCORES

    order = np.lexsort((sp, dp))
    sp, dp = sp[order], dp[order]
    core_of, kpos_of = core_of[order], kpos_of[order]
    hrow, in_a = hrow[order], in_a[order]
    dloc = dp % 128

    # fp8e4 x-stream: rms-normalize so values sit in e4m3's sweet spot;
    # the scale is undone inside W0 (aggregation is linear).
    x_pre = x * dinv[:, None]
    sx = float(1.0 / np.sqrt((x_pre ** 2).mean()))
    x_pre_pos = np.zeros((NPAD + 1, 128), ml_dtypes.float8_e4m3)
    x_pre_pos[pos] = (x_pre * sx).astype(ml_dtypes.float8_e4m3)
    dinv_pos = np.zeros(NPAD, np.float32)
    dinv_pos[pos] = dinv

    # ---- per-(core, position) edge groups ----
    EB = [[None] * TPC for _ in range(NCORES)]
    for c in range(NCORES):
        mc = core_of == c
        spc, kc, dl, hr, ia = sp[mc], kpos_of[mc], dloc[mc], hrow[mc], in_a[mc]
        for k in range(TPC):
            mk = kc == k
            EB[c][k] = (hr[mk], dl[mk], spc[mk], ia[mk])

    # per-position chunk counts (max over cores -> same program everywhere)
    CCA, CCB, CK, CA = [], [], [], []
    for k in range(TPC):
        cca = ccb = ca = 0
        for c in range(NCORES):
            hr, dl, _, ia = EB[c][k]
            na = int(ia.sum())
            nb = len(hr) - na
            cca = max(cca, -(-na // 128))
            ccb = max(ccb, -(-nb // 128))
            if len(dl):
                ca = max(ca, int(np.bincount(dl, minlength=128).max()))
        CCA.append(max(cca, 1))
        CCB.append(max(ccb, 1))
        CK.append(CCA[-1] + CCB[-1])
        ca = max(ca, 2)
        CA.append(ca + (ca & 1))             # even, for DoubleRow pairs
    SCA, SCK, SA, SB = sum(CA), sum(CK), sum(CCA), sum(CCB)

    ident_np = np.eye(128, dtype=np.float16)
    ident8_np = np.tile(np.eye(128), (1, 2)).astype(ml_dtypes.float8_e4m3)
    iota_np = np.tile(np.arange(128, dtype=np.float16), (128, 1))
    has_b0 = bool(np.any(b0))
    has_b12 = bool(np.any(b1)) or bool(np.any(b2))

    in_maps = []
    for c in range(NCORES):
        xe = np.zeros((128, SCA * 128), ml_dtypes.float8_e4m3)
        m1 = np.zeros((TPC, 128, 128), np.float16)
        m2 = np.zeros((TPC, 128, 128), np.float16) if has_b0 else None
        ia_arr = np.zeros((128, SA * 8), np.int16)
        ib_arr = np.zeros((128, SB * 8), np.int16)
        dk_arr = np.full((128, SCK), 255.0, np.float16)
        dinvp = np.zeros((128, TPC), np.float32)
        acol = bcol = kcol = xcol = 0
        for k in range(TPC):
            hr, dl, spk, iam = EB[c][k]
            ca, cca, ccb, ck = CA[k], CCA[k], CCB[k], CK[k]

            # agg0 stream: [128 nodes, ca slots, 128 f], pads -> zero row
            blk = np.full((128, ca), NPAD, np.int64)
            if len(dl):
                starts = np.concatenate(
                    [[0], np.flatnonzero(np.diff(dl)) + 1])
                lens = np.diff(np.concatenate([starts, [len(dl)]]))
                j_idx = np.arange(len(dl)) - np.repeat(starts, lens)
                blk[dl, j_idx] = spk
            xe[:, xcol * 128:(xcol + ca) * 128] = \
                x_pre_pos[blk.ravel()].reshape(128, ca * 128)
            xcol += ca

            # agg1 gather metadata: group-A slots then group-B slots
            hr_a, dl_a = hr[iam], dl[iam]
            hr_b, dl_b = hr[~iam], dl[~iam]
            iaw = np.zeros(cca * 128, np.int16)
            iaw[:len(hr_a)] = hr_a.astype(np.int16)
            ibw = np.zeros(ccb * 128, np.int16)
            ibw[:len(hr_b)] = hr_b.astype(np.int16)
            ia_arr[:, acol * 8:(acol + cca) * 8] = _wrap_idx16(iaw)
            ib_arr[:, bcol * 8:(bcol + ccb) * 8] = _wrap_idx16(ibw)
            acol += cca
            bcol += ccb

            # dst-row stream for on-chip one-hot build (255 = pad)
            dkk = np.full(ck * 128, 255, np.int64)
            dkk[:len(dl_a)] = dl_a
            dkk[cca * 128:cca * 128 + len(dl_b)] = dl_b
            dk_arr[:, kcol:kcol + ck] = \
                dkk.reshape(ck, 128).T.astype(np.float16)
            kcol += ck

            nodes_pos = (k * NCORES + c) * 128 + np.arange(128)
            real = nodes_pos < N
            pn = perm[np.clip(nodes_pos, 0, N - 1)]
            dinvp[:, k] = dinv_pos[nodes_pos]
            m1k = drop_mask[pn] * (dinv[pn] ** 2)[:, None]
            m1k[~real] = 0.0
            m1[k] = m1k.astype(np.float16)
            if has_b0:
                m2k = drop_mask[pn] * b0[None, :] * dinv[pn][:, None]
                m2k[~real] = 0.0
                m2[k] = m2k.astype(np.float16)

        im = {"xe": xe, "m1": m1, "ia": ia_arr, "ib": ib_arr, "dk": dk_arr,
              "dinvp": dinvp, "ident": ident_np, "ident82": ident8_np,
              "iota": iota_np,
              "w0t": np.ascontiguousarray(W0.T / sx).astype(np.float16),
              "w1t": np.ascontiguousarray(W1.T),
              "w2t": np.ascontiguousarray(W2.T)}
        if has_b0:
            im["m2"] = m2
        if has_b12:
            im["b1b"] = np.tile(b1, (128, 1))
            im["b2b"] = np.tile(b2, (128, 1))
        in_maps.append(im)

    nc = _build_kernel(CCA, CCB, CK, CA, has_b0, has_b12)
    res = run_bass_kernel_spmd(
        nc, in_maps, core_ids=list(range(NCORES)),
        trace=(os.environ.get("KTRACE", "0") == "1"))
    kernel.last_result = res

    out1 = np.zeros((NPAD, 128), np.float32)
    out2 = np.zeros((NPAD, 128), np.float32)
    for c in range(NCORES):
        r1 = res.results[c]["o1"].reshape(NPC, 128).astype(np.float32)
        r2 = res.results[c]["o2"].reshape(NPC, 128).astype(np.float32)
        for k in range(TPC):
            t = k * NCORES + c
            out1[t * 128:(t + 1) * 128] = r1[k * 128:(k + 1) * 128]
            out2[t * 128:(t + 1) * 128] = r2[k * 128:(k + 1) * 128]
    return out1[pos], out2[pos]



# revision 58
# speedup vs baseline: 1.0846x; 1.0846x over previous
"""GCN encoder kernel for 8 Trainium2 NeuronCores (Bass/Tile, SPMD).

Strategy (dst-sharded graph parallel, per sharding hint):
  - Nodes are degree-sorted and padded to NPAD = 392 tiles of 128; tiles go
    round-robin to the 8 cores so every core sees the same per-position
    chunk-count profile (SPMD: one program, 8 in_maps).
  - Aggregation is linear, so each GCN layer is computed as
    (aggregate) @ W.T; layers 1 and 2 share ONE aggregation of h.
  - agg0 (over x): the host expands x*dinv[src] into a padded per-(node,slot)
    edge-feature stream (pure data movement / sharding prep); the device
    reduces it with PE matmuls against a constant identity (PSUM scatter-add).
  - agg1 (over h): the device gathers h rows with dma_gather (4 SWDGE
    queues round-robin, buffers deep enough for 2 tiles in flight), builds
    exact 0/1 one-hot S matrices ON-CHIP via DVE is_equal against a resident
    iota tile (dst indices stream in as a tiny resident f16 array), and
    scatter-adds with PE matmuls: psum[f,d] += G.T @ S.
  - h is exchanged in TWO AllGather collectives (first 25 tiles, last 24),
    each issued as soon as its half of h is ready -> mostly hidden under
    phase A.  Side effect: each gathered tensor has <32768 rows, so int16
    gather indices need no lo/hi range splitting.
  - All gather indices + dst streams are SBUF-resident (loaded once).
  - Symmetric normalization (dinv = 1/sqrt(deg+1)) is folded into host-side
    scale arrays and a per-partition output scale; self-loops are ordinary
    edges.  Outputs are written f16 and upcast on the host.
"""
import os
import sys

sys.path.insert(0, "/opt/trn_rl_repo")

import numpy as np

N, E, DIN, DH = 50000, 1600000, 128, 128
NCORES = 8
NPAD = ((N + 1023) // 1024) * 1024   # 50176 = 392 tiles of 128
TILES = NPAD // 128
TPC = TILES // NCORES                # positions (tiles) per core
NPC = TPC * 128                      # node rows per core
KA = 25                              # tiles in first AllGather half
KB = TPC - KA                        # tiles in second half
NRA = NCORES * KA * 128              # 25600 rows in h_fullA (< 32768)
NRB = NCORES * KB * 128              # 24576 rows in h_fullB (< 32768)


def _wrap_idx16(a):
    """dma_gather index layout: idx i -> [i%16, i//16], replicated 8x."""
    n = len(a)
    w = np.zeros((16, n // 16), np.int16)
    w[np.arange(n) % 16, np.arange(n) // 16] = a
    return np.tile(w, (8, 1))


def _build_kernel(CCA, CCB, CK, CA, has_b0, has_b12):
    """Build the SPMD Tile program. CCA/CCB/CK/CA are per-position chunk
    counts (compile-time constants, shared by all cores)."""
    import concourse.bass as bass  # noqa: F401
    import concourse.tile as tile
    from concourse import bacc, mybir

    f32, f16, i16 = mybir.dt.float32, mybir.dt.float16, mybir.dt.int16
    f8 = mybir.dt.float8e4
    SCA, SCK = sum(CA), sum(CK)
    SA, SB = sum(CCA), sum(CCB)
    CAmax, CKmax = max(CA), max(CK)

    nc = bacc.Bacc(None, target_bir_lowering=False, debug=False,
                   num_swdge_queues=4)

    xe_d = nc.dram_tensor("xe", [128, SCA * 128], f8, kind="ExternalInput")
    m1_d = nc.dram_tensor("m1", [TPC, 128, 128], f16, kind="ExternalInput")
    ia_d = nc.dram_tensor("ia", [128, SA * 8], i16, kind="ExternalInput")
    ib_d = nc.dram_tensor("ib", [128, SB * 8], i16, kind="ExternalInput")
    dk_d = nc.dram_tensor("dk", [128, SCK], f16, kind="ExternalInput")
    dinv_d = nc.dram_tensor("dinvp", [128, TPC], f32, kind="ExternalInput")
    ident_d = nc.dram_tensor("ident", [128, 128], f16, kind="ExternalInput")
    ident8_d = nc.dram_tensor("ident82", [128, 256], f8, kind="ExternalInput")
    iota_d = nc.dram_tensor("iota", [128, 128], f16, kind="ExternalInput")
    w0_d = nc.dram_tensor("w0t", [128, 128], f16, kind="ExternalInput")
    w1_d = nc.dram_tensor("w1t", [128, 128], f32, kind="ExternalInput")
    w2_d = nc.dram_tensor("w2t", [128, 128], f32, kind="ExternalInput")
    if has_b0:
        m2_d = nc.dram_tensor("m2", [TPC, 128, 128], f16, kind="ExternalInput")
    if has_b12:
        b1_d = nc.dram_tensor("b1b", [128, 128], f32, kind="ExternalInput")
        b2_d = nc.dram_tensor("b2b", [128, 128], f32, kind="ExternalInput")
    o1_d = nc.dram_tensor("o1", [TPC, 128, 128], f16, kind="ExternalOutput")
    o2_d = nc.dram_tensor("o2", [TPC, 128, 128], f16, kind="ExternalOutput")

    with tile.TileContext(nc) as tc:
        with (
            tc.tile_pool(name="const", bufs=1) as cpool,
            tc.tile_pool(name="acc", bufs=1) as apool,
            tc.tile_pool(name="dram", bufs=1, space="DRAM") as dpool,
        ):
            ident_sb = cpool.tile([128, 128], f16)
            nc.scalar.dma_start(ident_sb[:], ident_d[:])
            ident8_sb = cpool.tile([128, 256], f8)
            nc.scalar.dma_start(ident8_sb[:], ident8_d[:])
            iota_sb = cpool.tile([128, 128], f16)
            nc.scalar.dma_start(iota_sb[:], iota_d[:])
            w0_sb = cpool.tile([128, 128], f16)
            nc.scalar.dma_start(w0_sb[:], w0_d[:])
            w1_sb = cpool.tile([128, 128], f32)
            nc.scalar.dma_start(w1_sb[:], w1_d[:])
            w2_sb = cpool.tile([128, 128], f32)
            nc.scalar.dma_start(w2_sb[:], w2_d[:])
            dinv_sb = cpool.tile([128, TPC], f32)
            nc.scalar.dma_start(dinv_sb[:], dinv_d[:])
            ia_sb = cpool.tile([128, SA * 8], i16)
            ib_sb = cpool.tile([128, SB * 8], i16)
            dk_sb = cpool.tile([128, SCK], f16)
            if has_b12:
                b1_sb = cpool.tile([128, 128], f32)
                nc.scalar.dma_start(b1_sb[:], b1_d[:])
                b2_sb = cpool.tile([128, 128], f32)
                nc.scalar.dma_start(b2_sb[:], b2_d[:])

            h_sbA = apool.tile([128, KA, 128], f16)
            h_sbB = apool.tile([128, KB, 128], f16)
            m1r_sb = apool.tile([128, TPC, 128], f16)
            nc.scalar.dma_start(
                m1r_sb[:], m1_d[:].rearrange("t p f -> p t f"))

            h_locA = dpool.tile([KA, 128, 128], f16)
            h_locB = dpool.tile([KB, 128, 128], f16)
            h_fullA = dpool.tile([NRA, 128], f16, addr_space="Shared")
            h_fullB = dpool.tile([NRB, 128], f16, addr_space="Shared")

            # ---------------- Phase A: agg0 + h ----------------
            with (
                tc.tile_pool(name="xe", bufs=3) as xpool,
                tc.tile_pool(name="meta", bufs=3) as mpool,
                tc.tile_pool(name="y", bufs=3) as ypool,
                tc.tile_pool(name="ps", bufs=2, space="PSUM") as ppool,
                tc.tile_pool(name="ps2", bufs=2, space="PSUM") as ppool2,
            ):
                # the W0 matmul of tile k-1 is emitted AFTER tile k's psum
                # accumulation: the PE then never waits on the scalar copy
                # and the instruction stream stays dense (phase A is
                # PE-issue-bound, not FLOP-bound).
                def w0_tail(k, y_sb):
                    ps_h = ppool2.tile([128, 128], f32, tag="ph")
                    nc.tensor.matmul(ps_h[:], y_sb[:], w0_sb[:],
                                     start=True, stop=True)  # [n, f2]
                    tmp = ypool.tile([128, 128], f32, tag="tmp")
                    nc.vector.tensor_tensor(tmp[:], ps_h[:], m1r_sb[:, k, :],
                                            mybir.AluOpType.mult)
                    if has_b0:
                        m2_sb = mpool.tile([128, 128], f16, tag="m2")
                        nc.sync.dma_start(m2_sb[:], m2_d[k])
                        nc.vector.tensor_tensor(tmp[:], tmp[:], m2_sb[:],
                                                mybir.AluOpType.add)
                    if k < KA:
                        nc.scalar.activation(h_sbA[:, k, :], tmp[:],
                                             mybir.ActivationFunctionType.Relu)
                    else:
                        nc.scalar.activation(h_sbB[:, k - KA, :], tmp[:],
                                             mybir.ActivationFunctionType.Relu)

                acol = 0
                pend = None              # (k, y_sb) awaiting its W0 tail
                for k in range(TPC):
                    ca = CA[k]           # even by construction
                    ca2 = ca // 2
                    gt = xpool.tile([128, CAmax // 2, 256], f8, tag="gt")
                    nc.sync.dma_start(
                        gt[:, 0:ca2, :].rearrange("p c f -> p (c f)"),
                        xe_d[:, acol * 128:(acol + ca) * 128])
                    acol += ca

                    # fp8e4 DoubleRow: each matmul contracts TWO slots
                    # (lhsT = [slot2j | slot2j+1], rhs = [I | I]), halving
                    # the PE instruction count.
                    ps_a = ppool.tile([128, 128], f32, tag="pa")
                    id2 = ident8_sb[:].rearrange("p (two f) -> p two f", two=2)
                    for j in range(ca2):
                        nc.tensor.matmul(ps_a[:],
                                         gt[:, j, :].rearrange(
                                             "p (two f) -> p two f", two=2),
                                         id2,
                                         start=(j == 0), stop=(j == ca2 - 1),
                                         perf_mode=mybir.MatmulPerfMode.DoubleRow)
                    y_sb = ypool.tile([128, 128], f16, tag="y0")
                    nc.scalar.copy(y_sb[:], ps_a[:])        # [f, d] raw sums
                    if pend is not None:
                        w0_tail(*pend)
                    pend = (k, y_sb)

                    if k == 1:
                        nc.scalar.dma_start(ia_sb[:], ia_d[:])
                        nc.scalar.dma_start(ib_sb[:], ib_d[:])
                        nc.scalar.dma_start(dk_sb[:], dk_d[:])
                    if k == KA:
                        # tile KA-1's tail ran above, so h_sbA is complete
                        nc.sync.dma_start(
                            h_locA[:].rearrange("t p f -> p t f"), h_sbA[:])
                        nc.gpsimd.collective_compute(
                            "AllGather", mybir.AluOpType.bypass,
                            replica_groups=[list(range(NCORES))],
                            ins=[h_locA[:]], outs=[h_fullA[:]],
                        )
                w0_tail(*pend)
                nc.sync.dma_start(
                    h_locB[:].rearrange("t p f -> p t f"), h_sbB[:])

            # ---------------- Phase B: agg1 + outputs ----------------
            # Separate buffer rings for the A-half and B-half gathers, with
            # gather emission lagged behind buffer release so the gpsimd
            # queue never head-of-line blocks: each dma_gather's buffer was
            # freed two tile-periods earlier.  Queues rotate per call.
            RA, RB = 10, 6
            PREA, PREB = 10, 4
            qctr = [0]
            offA = np.cumsum([0] + CCA).tolist()
            offB = np.cumsum([0] + CCB).tolist()
            CCAmax, CCBmax = max(CCA), max(CCB)
            GAt = [None] * TPC
            GBt = [None] * TPC
            with (
                tc.tile_pool(name="ga", bufs=RA) as gapool,
                tc.tile_pool(name="gb", bufs=RB) as gbpool,
                tc.tile_pool(name="s", bufs=2) as spool,
                tc.tile_pool(name="y2", bufs=2) as ypool2,
                tc.tile_pool(name="o", bufs=2) as opool,
                tc.tile_pool(name="psb", bufs=2, space="PSUM") as ppoolb,
                tc.tile_pool(name="psb2", bufs=2, space="PSUM") as ppoolb2,
            ):
                def a_gather(k):
                    cca = CCA[k]
                    GA = gapool.tile([128, CCAmax, 128], f16, tag="GA")
                    GAt[k] = GA
                    nc.gpsimd.dma_gather(GA[:, 0:cca, :], h_fullA[:],
                                         ia_sb[:, offA[k] * 8:(offA[k] + cca) * 8],
                                         cca * 128, cca * 128, 128,
                                         elem_step=128, single_packet=False,
                                         queue_num=qctr[0] % 4)
                    qctr[0] += 1

                def b_gather(k):
                    ccb = CCB[k]
                    GB = gbpool.tile([128, CCBmax, 128], f16, tag="GB")
                    GBt[k] = GB
                    nc.gpsimd.dma_gather(GB[:, 0:ccb, :], h_fullB[:],
                                         ib_sb[:, offB[k] * 8:(offB[k] + ccb) * 8],
                                         ccb * 128, ccb * 128, 128,
                                         elem_step=128, single_packet=False,
                                         queue_num=qctr[0] % 4)
                    qctr[0] += 1

                for k in range(min(PREA, TPC)):
                    a_gather(k)
                nc.gpsimd.collective_compute(
                    "AllGather", mybir.AluOpType.bypass,
                    replica_groups=[list(range(NCORES))],
                    ins=[h_locB[:]], outs=[h_fullB[:]],
                )
                for k in range(min(PREB, TPC)):
                    b_gather(k)

                kcol = 0
                for k in range(TPC):
                    cca, ccb, ck = CCA[k], CCB[k], CK[k]
                    if k % 2 == 0:
                        for kk in (k + PREA, k + PREA + 1):
                            if kk < TPC and kk >= PREA:
                                a_gather(kk)
                        for kk in (k + PREB, k + PREB + 1):
                            if kk < TPC and kk >= PREB:
                                b_gather(kk)

                    S = spool.tile([128, CKmax, 128], f16, tag="S")
                    iota_bc = iota_sb[:].rearrange(
                        "p (o f) -> p o f", o=1).broadcast_to([128, ck, 128])
                    dk_bc = dk_sb[:, kcol:kcol + ck].rearrange(
                        "p (c o) -> p c o", o=1).broadcast_to([128, ck, 128])
                    nc.vector.tensor_tensor(
                        S[:, 0:ck, :], iota_bc, dk_bc,
                        mybir.AluOpType.is_equal)
                    kcol += ck

                    GA, GB = GAt[k], GBt[k]
                    ps_b = ppoolb.tile([128, 128], f32, tag="pb")
                    for c in range(cca):
                        nc.tensor.matmul(ps_b[:], GA[:, c, :], S[:, c, :],
                                         start=(c == 0), stop=False)
                    for c in range(ccb):
                        nc.tensor.matmul(ps_b[:], GB[:, c, :],
                                         S[:, cca + c, :],
                                         start=False, stop=(c == ccb - 1))
                    y2 = ypool2.tile([128, 128], f32, tag="y2")
                    nc.scalar.copy(y2[:], ps_b[:])          # [f, d] raw sums

                    ps_o1 = ppoolb2.tile([128, 128], f32, tag="po")
                    nc.tensor.matmul(ps_o1[:], y2[:], w1_sb[:],
                                     start=True, stop=True)
                    ps_o2 = ppoolb2.tile([128, 128], f32, tag="po")
                    nc.tensor.matmul(ps_o2[:], y2[:], w2_sb[:],
                                     start=True, stop=True)

                    o1t = opool.tile([128, 128], f16, tag="o1")
                    nc.scalar.activation(o1t[:], ps_o1[:],
                                         mybir.ActivationFunctionType.Copy,
                                         scale=dinv_sb[:, k:k + 1])
                    o2t = opool.tile([128, 128], f16, tag="o2")
                    nc.scalar.activation(o2t[:], ps_o2[:],
                                         mybir.ActivationFunctionType.Copy,
                                         scale=dinv_sb[:, k:k + 1])
                    if has_b12:
                        nc.vector.tensor_tensor(o1t[:], o1t[:], b1_sb[:],
                                                mybir.AluOpType.add)
                        nc.vector.tensor_tensor(o2t[:], o2t[:], b2_sb[:],
                                                mybir.AluOpType.add)
                    nc.sync.dma_start(o1_d[k], o1t[:])
                    nc.sync.dma_start(o2_d[k], o2t[:])

    nc.compile()
    return nc


def kernel(x, edge_index, drop_mask, W0, b0, W1, b1, W2, b2, **_):
    import ml_dtypes
    from concourse.bass_utils import run_bass_kernel_spmd

    x = np.asarray(x, np.float32)
    edge_index = np.asarray(edge_index)
    drop_mask = np.asarray(drop_mask, np.float32)
    W0, W1, W2 = (np.asarray(w, np.float32) for w in (W0, W1, W2))
    b0, b1, b2 = (np.asarray(b, np.float32) for b in (b0, b1, b2))
    src0, dst0 = edge_index[0].astype(np.int64), edge_index[1].astype(np.int64)

    # ---- normalization / permutation (host: index-side preprocessing) ----
    deg = np.bincount(dst0, minlength=N).astype(np.float32) + 1.0
    dinv = 1.0 / np.sqrt(deg)

    perm = np.argsort(-deg, kind="stable")           # position -> node id
    pos = np.empty(N, np.int64)                      # node id -> position
    pos[perm] = np.arange(N)

    # self loops as ordinary edges
    src_a = np.concatenate([src0, np.arange(N)])
    dst_a = np.concatenate([dst0, np.arange(N)])
    sp = pos[src_a]
    dp = pos[dst_a]

    # h storage row of a source position, split across the two AllGathers:
    #   tile t = p//128 -> core t%8, per-core tile index t//8
    #   first KA per-core tiles -> h_fullA, rest -> h_fullB
    st = sp // 128
    sk = st // NCORES
    in_a = sk < KA
    hrow = np.where(
        in_a,
        (st % NCORES) * (KA * 128) + sk * 128 + (sp % 128),
        (st % NCORES) * (KB * 128) + (sk - KA) * 128 + (sp % 128))

    tile_of = dp // 128
    core_of = tile_of % NCORES
    kpos_of = tile_of // NCORES

    order = np.lexsort((sp, dp))
    sp, dp = sp[order], dp[order]
    core_of, kpos_of = core_of[order], kpos_of[order]
    hrow, in_a = hrow[order], in_a[order]
    dloc = dp % 128

    # fp8e4 x-stream: rms-normalize so values sit in e4m3's sweet spot;
    # the scale is undone inside W0 (aggregation is linear).
    x_pre = x * dinv[:, None]
    sx = float(1.0 / np.sqrt((x_pre ** 2).mean()))
    x_pre_pos = np.zeros((NPAD + 1, 128), ml_dtypes.float8_e4m3)
    x_pre_pos[pos] = (x_pre * sx).astype(ml_dtypes.float8_e4m3)
    dinv_pos = np.zeros(NPAD, np.float32)
    dinv_pos[pos] = dinv

    # ---- per-(core, position) edge groups ----
    EB = [[None] * TPC for _ in range(NCORES)]
    for c in range(NCORES):
        mc = core_of == c
        spc, kc, dl, hr, ia = sp[mc], kpos_of[mc], dloc[mc], hrow[mc], in_a[mc]
        for k in range(TPC):
            mk = kc == k
            EB[c][k] = (hr[mk], dl[mk], spc[mk], ia[mk])

    # per-position chunk counts (max over cores -> same program everywhere)
    CCA, CCB, CK, CA = [], [], [], []
    for k in range(TPC):
        cca = ccb = ca = 0
        for c in range(NCORES):
            hr, dl, _, ia = EB[c][k]
            na = int(ia.sum())
            nb = len(hr) - na
            cca = max(cca, -(-na // 128))
            ccb = max(ccb, -(-nb // 128))
            if len(dl):
                ca = max(ca, int(np.bincount(dl, minlength=128).max()))
        CCA.append(max(cca, 1))
        CCB.append(max(ccb, 1))
        CK.append(CCA[-1] + CCB[-1])
        ca = max(ca, 2)
        CA.append(ca + (ca & 1))             # even, for DoubleRow pairs
    SCA, SCK, SA, SB = sum(CA), sum(CK), sum(CCA), sum(CCB)

    ident_np = np.eye(128, dtype=np.float16)
    ident8_np = np.tile(np.eye(128), (1, 2)).astype(ml_dtypes.float8_e4m3)
    iota_np = np.tile(np.arange(128, dtype=np.float16), (128, 1))
    has_b0 = bool(np.any(b0))
    has_b12 = bool(np.any(b1)) or bool(np.any(b2))

    in_maps = []
    for c in range(NCORES):
        xe = np.zeros((128, SCA * 128), ml_dtypes.float8_e4m3)
        m1 = np.zeros((TPC, 128, 128), np.float16)
        m2 = np.zeros((TPC, 128, 128), np.float16) if has_b0 else None
        ia_arr = np.zeros((128, SA * 8), np.int16)
        ib_arr = np.zeros((128, SB * 8), np.int16)
        dk_arr = np.full((128, SCK), 255.0, np.float16)
        dinvp = np.zeros((128, TPC), np.float32)
        acol = bcol = kcol = xcol = 0
        for k in range(TPC):
            hr, dl, spk, iam = EB[c][k]
            ca, cca, ccb, ck = CA[k], CCA[k], CCB[k], CK[k]

            # agg0 stream: [128 nodes, ca slots, 128 f], pads -> zero row
            blk = np.full((128, ca), NPAD, np.int64)
            if len(dl):
                starts = np.concatenate(
                    [[0], np.flatnonzero(np.diff(dl)) + 1])
                lens = np.diff(np.concatenate([starts, [len(dl)]]))
                j_idx = np.arange(len(dl)) - np.repeat(starts, lens)
                blk[dl, j_idx] = spk
            xe[:, xcol * 128:(xcol + ca) * 128] = \
                x_pre_pos[blk.ravel()].reshape(128, ca * 128)
            xcol += ca

            # agg1 gather metadata: group-A slots then group-B slots
            hr_a, dl_a = hr[iam], dl[iam]
            hr_b, dl_b = hr[~iam], dl[~iam]
            iaw = np.zeros(cca * 128, np.int16)
            iaw[:len(hr_a)] = hr_a.astype(np.int16)
            ibw = np.zeros(ccb * 128, np.int16)
            ibw[:len(hr_b)] = hr_b.astype(np.int16)
            ia_arr[:, acol * 8:(acol + cca) * 8] = _wrap_idx16(iaw)
            ib_arr[:, bcol * 8:(bcol + ccb) * 8] = _wrap_idx16(ibw)
            acol += cca
            bcol += ccb

            # dst-row stream for on-chip one-hot build (255 = pad)
            dkk = np.full(ck * 128, 255, np.int64)
            dkk[:len(dl_a)] = dl_a
            dkk[cca * 128:cca * 128 + len(dl_b)] = dl_b
            dk_arr[:, kcol:kcol + ck] = \
                dkk.reshape(ck, 128).T.astype(np.float16)
            kcol += ck

            nodes_pos = (k * NCORES + c) * 128 + np.arange(128)
            real = nodes_pos < N
            pn = perm[np.clip(nodes_pos, 0, N - 1)]
            dinvp[:, k] = dinv_pos[nodes_pos]
            m1k = drop_mask[pn] * (dinv[pn] ** 2)[:, None]
            m1k[~real] = 0.0
            m1[k] = m1k.astype(np.float16)
            if has_b0:
                m2k = drop_mask[pn] * b0[None, :] * dinv[pn][:, None]
                m2k[~real] = 0.0
                m2[k] = m2k.astype(np.float16)

        im = {"xe": xe, "m1": m1, "ia": ia_arr, "ib": ib_arr, "dk": dk_arr,
              "dinvp": dinvp, "ident": ident_np, "ident82": ident8_np,
              "iota": iota_np,
              "w0t": np.ascontiguousarray(W0.T / sx).astype(np.float16),
              "w1t": np.ascontiguousarray(W1.T),
              "w2t": np.ascontiguousarray(W2.T)}
        if has_b0:
            im["m2"] = m2
        if has_b12:
            im["b1b"] = np.tile(b1, (128, 1))
            im["b2b"] = np.tile(b2, (128, 1))
        in_maps.append(im)

    nc = _build_kernel(CCA, CCB, CK, CA, has_b0, has_b12)
    res = run_bass_kernel_spmd(
        nc, in_maps, core_ids=list(range(NCORES)),
        trace=(os.environ.get("KTRACE", "0") == "1"))
    kernel.last_result = res

    out1 = np.zeros((NPAD, 128), np.float32)
    out2 = np.zeros((NPAD, 128), np.float32)
    for c in range(NCORES):
        r1 = res.results[c]["o1"].reshape(NPC, 128).astype(np.float32)
        r2 = res.results[c]["o2"].reshape(NPC, 128).astype(np.float32)
        for k in range(TPC):
            t = k * NCORES + c
            out1[t * 128:(t + 1) * 128] = r1[k * 128:(k + 1) * 128]
            out2[t * 128:(t + 1) * 128] = r2[k * 128:(k + 1) * 128]
    return out1[pos], out2[pos]

